# revision 11
# baseline (speedup 1.0000x reference)
"""AddAttention kernel for Trainium2, SPMD across 8 NeuronCores.

Math: score[b,i,j] = sq[b,i] + sk[b,j] with sq = inp@wq, sk = inp@wk.
softmax over j cancels the sq[b,i] term exactly, so
    attn[b,i,:] = softmax(sk[b,:])   (identical for every i)
    out[b,i,:]  = softmax(sk[b,:]) @ inp[b]   (identical for every i)
The kernel computes the 1-D softmax per batch and broadcast-writes the
replicated rows. Data-parallel over batch: 2 batches per core.
"""

import numpy as np

import concourse.bacc as bacc
import concourse.mybir as mybir
from concourse.tile import TileContext
from concourse.bass_utils import run_bass_kernel_spmd

B, L, D = 16, 2048, 256
NCORES = 8
BPC = B // NCORES  # batches per core
P = 128
NCH = L // P  # 16 row-chunks of 128

F32 = mybir.dt.float32

# Set to True to emit single broadcast (stride-0 source) DMAs for the
# replicated-row writes; False falls back to one DMA per 128-row chunk.
BROADCAST_DMA = True


def build_bass(repeat=1):
    nc = bacc.Bacc()
    inp = nc.declare_dram_parameter("inp", [BPC, L, D], F32, isOutput=False)
    v_w = nc.declare_dram_parameter("v_w", [1, 2 * D], F32, isOutput=False)
    out = nc.declare_dram_parameter("out", [BPC, L, D], F32, isOutput=True)
    attn = nc.declare_dram_parameter("attn", [BPC, L, L], F32, isOutput=True)

    from concourse.masks import make_identity

    with TileContext(nc) as tc:
        with (
            tc.tile_pool(name="consts", bufs=1) as cpool,
            tc.tile_pool(name="io", bufs=2) as iopool,
            tc.tile_pool(name="small", bufs=2) as spool,
            tc.tile_pool(name="psum_row", bufs=1, space="PSUM") as prow,
            tc.tile_pool(name="psum_small", bufs=4, space="PSUM") as psmall,
        ):
            # constants
            wk_b = cpool.tile([P, D], F32)  # wk broadcast to all partitions
            nc.sync.dma_start(
                out=wk_b[:], in_=v_w[0:1, D : 2 * D].broadcast_to([P, D])
            )
            ones_row = cpool.tile([1, P], F32)
            nc.vector.memset(ones_row[:], 1.0)
            ones_col = cpool.tile([P, 1], F32)
            nc.vector.memset(ones_col[:], 1.0)
            ident = cpool.tile([P, P], F32)
            make_identity(nc, ident[:])

            for b in [b for _ in range(repeat) for b in range(BPC)]:
                # ---- load inp[b] into [128, 16, 256] (partition = row % 128)
                inp_sb = iopool.tile([P, NCH, D], F32, tag="inp_sb")
                # loads go on the ACT HWDGE ring so they are not FIFO-queued
                # behind the 16 MB attn writes on the SP ring
                LOADS = 4  # chunks per DMA
                for g in range(NCH // LOADS):
                    c0 = g * LOADS
                    nc.scalar.dma_start(
                        out=inp_sb[:, c0 : c0 + LOADS, :],
                        in_=inp[b, c0 * P : (c0 + LOADS) * P, :].rearrange(
                            "(c p) d -> p c d", p=P
                        ),
                    )

                # ---- sk[p, c] = <inp row, wk>
                sk = spool.tile([P, NCH], F32, tag="sk")
                prod = spool.tile([P, D], F32, tag="prod")
                for c in range(NCH):
                    nc.vector.tensor_tensor(
                        out=prod[:],
                        in0=inp_sb[:, c, :],
                        in1=wk_b[:],
                        op=mybir.AluOpType.mult,
                    )
                    nc.vector.tensor_reduce(
                        out=sk[:, c : c + 1],
                        in_=prod[:],
                        axis=mybir.AxisListType.X,
                        op=mybir.AluOpType.add,
                    )

                # ---- e = exp(sk)  (|sk| < ~5, no max-subtraction needed)
                # accum_out gives per-partition row sums for free.
                e_wide = spool.tile([P, NCH], F32, tag="e_wide")
                rowsum = spool.tile([P, 1], F32, tag="rowsum")
                nc.scalar.activation(
                    e_wide[:],
                    sk[:],
                    mybir.ActivationFunctionType.Exp,
                    accum_out=rowsum[:],
                )

                # ---- S = sum over partitions (ones matmul), rinv = 1/S
                psum_S = psmall.tile([1, 1], F32, tag="ps")
                nc.tensor.matmul(
                    psum_S[:], lhsT=ones_col[:], rhs=rowsum[:], start=True, stop=True
                )
                rinv = spool.tile([1, 1], F32, tag="rinv")
                nc.vector.reciprocal(rinv[:], psum_S[:])
                rinv_row = spool.tile([1, P], F32, tag="rinv_row")
                nc.vector.tensor_scalar_mul(rinv_row[:], ones_row[:], rinv[:])

                # ---- e_row[1, 2048] via 16 PE transposes of e_wide columns
                psum_erow = prow.tile([1, L], F32, tag="erow")
                for c in range(NCH):
                    nc.tensor.transpose(
                        psum_erow[0:1, c * P : (c + 1) * P],
                        e_wide[:, c : c + 1],
                        ident[:],
                    )
                e_row = spool.tile([1, L], F32, tag="e_row")
                nc.vector.tensor_copy(e_row[:], psum_erow[:])

                # ---- attn rows: [128, 2048] tile, every row = rinv * e_row
                attn_sb = iopool.tile([P, L], F32, tag="attn_sb")
                NB = 512  # matmul moving free dim / PSUM bank
                for n in range(L // NB):
                    psum_attn = psmall.tile([P, NB], F32, tag="ps")
                    nc.tensor.matmul(
                        psum_attn[:],
                        lhsT=rinv_row[:],
                        rhs=e_row[0:1, n * NB : (n + 1) * NB],
                        start=True,
                        stop=True,
                    )
                    nc.scalar.copy(attn_sb[:, n * NB : (n + 1) * NB], psum_attn[:])

                if BROADCAST_DMA:
                    nc.sync.dma_start(
                        out=attn[b].rearrange("(c p) j -> p c j", p=P),
                        in_=attn_sb[:].unsqueeze(1).broadcast_to([P, NCH, L]),
                    )
                else:
                    for c in range(NCH):
                        nc.sync.dma_start(
                            out=attn[b, c * P : (c + 1) * P, :], in_=attn_sb[:]
                        )

                # ---- row_out = e @ inp[b]  (K=128 per chunk, accumulate)
                psum_ro = psmall.tile([1, D], F32, tag="ps")
                for c in range(NCH):
                    nc.tensor.matmul(
                        psum_ro[:],
                        lhsT=e_wide[:, c : c + 1],
                        rhs=inp_sb[:, c, :],
                        start=(c == 0),
                        stop=(c == NCH - 1),
                    )
                ro_row = spool.tile([1, D], F32, tag="ro_row")
                nc.vector.tensor_copy(ro_row[:], psum_ro[:])

                # ---- out rows: [128, 256], every row = rinv * row_out
                psum_obc = psmall.tile([P, D], F32, tag="ps")
                nc.tensor.matmul(
                    psum_obc[:], lhsT=rinv_row[:], rhs=ro_row[:], start=True, stop=True
                )
                out_sb = spool.tile([P, D], F32, tag="out_sb")
                nc.vector.tensor_copy(out_sb[:], psum_obc[:])

                if BROADCAST_DMA:
                    nc.scalar.dma_start(
                        out=out[b].rearrange("(c p) d -> p c d", p=P),
                        in_=out_sb[:].unsqueeze(1).broadcast_to([P, NCH, D]),
                    )
                else:
                    for c in range(NCH):
                        nc.sync.dma_start(
                            out=out[b, c * P : (c + 1) * P, :], in_=out_sb[:]
                        )
    return nc


_NC_CACHE = None


def _get_nc():
    global _NC_CACHE
    if _NC_CACHE is None:
        nc = build_bass()
        if not nc.is_finalized():
            nc.finalize()
        _NC_CACHE = nc
    return _NC_CACHE


def kernel(inp, v_w, _trace=False, _result_box=None):
    inp = np.ascontiguousarray(inp, dtype=np.float32)
    v_w = np.ascontiguousarray(v_w, dtype=np.float32)
    nc = _get_nc()
    in_maps = [
        {"inp": inp[i * BPC : (i + 1) * BPC], "v_w": v_w} for i in range(NCORES)
    ]
    res = run_bass_kernel_spmd(nc, in_maps, core_ids=list(range(NCORES)), trace=_trace)
    if _result_box is not None:
        _result_box.append(res)
    out = np.concatenate([res.results[i]["out"] for i in range(NCORES)], axis=0)
    attn = np.concatenate([res.results[i]["attn"] for i in range(NCORES)], axis=0)
    return out, attn
